# revision 37
# baseline (speedup 1.0000x reference)
"""SupJSD / ContrastiveLossPlus loss kernel for 8 Trainium2 NeuronCores.

v3: fp16 data path; bn_stats for row norms; scale-free batched Ln(x);
amat (one-hot * s16) built on the Scalar engine; single fp16 matmul/tile.

Data arrives fp16, tile-transposed, in 16-tile groups (8KB/partition DMAs).
Per 128-row tile k (x >= 0, uniform [0,1)):
  bn_stats(x_k)  -> per-row (mean, var) of even/odd halves  (DVE, no accum)
    ss = cvar_e+cvar_o + 128*(me^2+mo^2)   sx = 128*(me+mo)   [group smalls]
  s16  = exp(-0.5*ln(ss)+ln16) = 16/||x||  (ACT Ln+Exp, batched /supergroup)
  lnx  = ln(x + 1e-30)                     (ACT Ln, batched per group,
                                            independent of the s16 chain)
  xlx_k = sum_d x*lnx                      (DVE stt accum)
  amat_k = onehot_k * s16                  (ACT Copy with per-row scale)
  psum += amat_k^T @ x                     (PE fp16 matmul -> 16*seg [80,256])
  u16 = s16*(xlx + (-0.5*ln(ss)+ln16)*sx)  [= 16*sum_d p*ln(16p), smalls]
End: e1 = sum_k u16[:,k]*W[:,k]  (W = 1/count per row, from host).
Host combines in float64:
  T1 = E1/16 - ln16 * sum_c (sum_d seg)/cnt
  loss = 0.01/D * (T1 - sum_c (1/cnt) sum_d seg*ln(clip(seg/cnt,eps)))
"""

import numpy as np

N_CORES = 8
N, D, C = 65536, 256, 80
R = 3 * N // N_CORES          # rows per core = 24576
T = R // 128                  # tiles per core = 192
G = 32                        # tiles per group (batched Ln)
NG = T // G                   # groups = 12
SG = 2                        # groups per supergroup (batched s16)
LOG16 = float(np.log(16.0))

_cache = {}


def _build_nc():
    from contextlib import ExitStack

    import concourse.tile as tile
    from concourse import bacc, mybir

    F32 = mybir.dt.float32
    F16 = mybir.dt.float16
    A = mybir.AluOpType
    ACTF = mybir.ActivationFunctionType

    nc = bacc.Bacc("TRN2", target_bir_lowering=False, debug=False,
                   num_devices=N_CORES)
    xin = nc.dram_tensor("xin", [128, T * D], F16, kind="ExternalInput").ap()
    ohin = nc.dram_tensor("ohin", [128, T * C], F16, kind="ExternalInput").ap()
    win = nc.dram_tensor("win", [128, T], F32, kind="ExternalInput").ap()
    out = nc.dram_tensor("acc", [C, D], F32, kind="ExternalOutput").ap()
    oute = nc.dram_tensor("e1", [128, 1], F32, kind="ExternalOutput").ap()

    with tile.TileContext(nc) as tc, ExitStack() as ctx:
        cpool = ctx.enter_context(tc.tile_pool(name="consts", bufs=1))
        xpool = ctx.enter_context(tc.tile_pool(name="x", bufs=4))
        mpool = ctx.enter_context(tc.tile_pool(name="m", bufs=3))
        lpool = ctx.enter_context(tc.tile_pool(name="lnm", bufs=4))
        opool = ctx.enter_context(tc.tile_pool(name="oh", bufs=NG + 1))
        spool = ctx.enter_context(tc.tile_pool(name="small", bufs=3))
        jpool = ctx.enter_context(tc.tile_pool(name="junk", bufs=2))
        rpool = ctx.enter_context(tc.tile_pool(name="res", bufs=1))
        pspool = ctx.enter_context(tc.tile_pool(name="ps", bufs=1, space="PSUM"))

        c_tiny24 = cpool.tile([128, 1], F32)
        nc.vector.memset(c_tiny24[:], 1e-24)
        c_tiny30 = cpool.tile([128, 1], F32)
        nc.vector.memset(c_tiny30[:], 1e-30)
        c_ln16 = cpool.tile([128, 1], F32)
        nc.vector.memset(c_ln16[:], LOG16)

        wt = cpool.tile([128, T], F32)
        nc.sync.dma_start(wt[:], win[:])

        e1cols = cpool.tile([128, NG], F32)
        junk1 = jpool.tile([128, D], F16, tag="junk")
        junk2 = jpool.tile([128, D], F16, tag="junk")
        junkg = jpool.tile([128, G], F32, tag="junkg")
        junkw = jpool.tile([128, NG], F32, tag="junkw")

        ps = pspool.tile([C, D], F32)

        xga, ohga = [], []
        for g in range(NG):
            xg = xpool.tile([128, G * D], F16, tag="xg")
            nch = 4 if g == 0 else 2
            W = G * D // nch
            for h in range(nch):
                nc.sync.dma_start(
                    xg[:, h * W:(h + 1) * W],
                    xin[:, g * G * D + h * W:g * G * D + (h + 1) * W])
            ohg = opool.tile([128, G * C], F16, tag="ohg")
            nc.sync.dma_start(ohg[:], ohin[:, g * G * C:(g + 1) * G * C])
            xga.append(xg)
            ohga.append(ohg)

        sg_sizes = [2, 2, 2]
        assert sum(sg_sizes) == NG
        sg_state = {}

        def emit_ss(sg, g_base, SGn):
            ssg = spool.tile([128, SG * G], F32, tag="ssg")
            sxg = spool.tile([128, SG * G], F32, tag="sxg")
            lnxs = []
            for g4 in range(SGn):
                g = g_base + g4
                xg = xga[g]
                # bn_stats per tile: (cnt,mean,cnt*var) of even/odd element
                # halves in one DVE pass each, no accumulator read needed.
                st = spool.tile([128, G * 6], F32, tag="bnst")
                for j in range(G):
                    nc.vector.bn_stats(st[:, j * 6:(j + 1) * 6],
                                       xg[:, j * D:(j + 1) * D])
                # lnx = ln(x + 1e-30), batched; independent of the s16 chain
                lnxg = lpool.tile([128, G * D], F16, tag="lnxg")
                H = G * D // 2
                nc.scalar.activation(lnxg[:, 0:H], xg[:, 0:H], ACTF.Ln,
                                     bias=c_tiny30[:])
                nc.scalar.activation(lnxg[:, H:], xg[:, H:], ACTF.Ln,
                                     bias=c_tiny30[:])
                lnxs.append(lnxg)
                # ss = cvar_e + cvar_o + (D/2)*(mean_e^2+mean_o^2); sx/128 = me+mo
                me, ve = st[:, 1::6], st[:, 2::6]
                mo, vo = st[:, 4::6], st[:, 5::6]
                sl = slice(g4 * G, (g4 + 1) * G)
                nc.vector.tensor_tensor(sxg[:, sl], me, mo, A.add)
                t1 = spool.tile([128, G], F32, tag="t1m")
                nc.vector.tensor_tensor(t1[:], me, me, A.mult)
                t2 = spool.tile([128, G], F32, tag="t2m")
                nc.vector.tensor_tensor(t2[:], mo, mo, A.mult)
                r = spool.tile([128, G], F32, tag="rm")
                nc.vector.tensor_tensor(r[:], ve, vo, A.add)
                t3 = spool.tile([128, G], F32, tag="t3m")
                nc.vector.scalar_tensor_tensor(
                    t3[:], t1[:], 1.0, t2[:], A.mult, A.add)
                nc.vector.scalar_tensor_tensor(
                    ssg[:, sl], t3[:], float(D // 2), r[:], A.mult, A.add)
            # s16 = exp(-0.5*ln(ss + 1e-24) + ln16) = 16/sqrt(ss)
            lss = spool.tile([128, SG * G], F32, tag="lss")
            nc.scalar.activation(lss[:, 0:SGn * G], ssg[:, 0:SGn * G],
                                 ACTF.Ln, bias=c_tiny24[:])
            s16g = spool.tile([128, SG * G], F32, tag="s16g")
            nc.scalar.activation(s16g[:, 0:SGn * G], lss[:, 0:SGn * G],
                                 ACTF.Exp, bias=c_ln16[:], scale=-0.5)
            sg_state[sg] = (lnxs, sxg, lss, s16g)

        def emit_m(sg, g_base, SGn):
            lnxs, sxg, lss, s16g = sg_state.pop(sg)
            for g4 in range(SGn):
                g = g_base + g4
                xg, ohg, lnxg = xga[g], ohga[g], lnxs[g4]
                sl = slice(g4 * G, (g4 + 1) * G)
                # amat = onehot * s16 (per-row), built on the Scalar engine
                am = mpool.tile([128, G * C], F16, tag="am")
                for j in range(G):
                    s16c = s16g[:, g4 * G + j:g4 * G + j + 1]
                    if g >= NG - 2:
                        nc.vector.tensor_scalar(
                            am[:, j * C:(j + 1) * C],
                            ohg[:, j * C:(j + 1) * C], s16c, None, A.mult)
                    else:
                        nc.scalar.activation(
                            am[:, j * C:(j + 1) * C], ohg[:, j * C:(j + 1) * C],
                            ACTF.Copy, scale=s16c)

                ug = spool.tile([128, G], F32, tag="ug")
                for j in range(G):
                    k = g * G + j
                    nc.vector.scalar_tensor_tensor(
                        junk2[:], xg[:, j * D:(j + 1) * D], 1.0,
                        lnxg[:, j * D:(j + 1) * D], A.mult, A.mult,
                        accum_out=ug[:, j:j + 1])
                    nc.tensor.matmul(ps[:], am[:, j * C:(j + 1) * C],
                                     xg[:, j * D:(j + 1) * D],
                                     start=(k == 0), stop=(k == T - 1))
                # u16 = s16*(xlx + lns16*sx); lns16 = -0.5*lss + ln16
                lns = spool.tile([128, G], F32, tag="lns")
                nc.vector.tensor_scalar(lns[:], lss[:, sl], -0.5, LOG16,
                                        A.mult, A.add)
                a1 = spool.tile([128, G], F32, tag="a1m")
                nc.vector.tensor_tensor(a1[:], lns[:], sxg[:, sl], A.mult)
                b1 = spool.tile([128, G], F32, tag="b1m")
                nc.vector.scalar_tensor_tensor(
                    b1[:], a1[:], float(D // 2), ug[:], A.mult, A.add)
                u16 = spool.tile([128, G], F32, tag="u16m")
                nc.vector.tensor_tensor(u16[:], b1[:], s16g[:, sl], A.mult)
                nc.vector.scalar_tensor_tensor(
                    junkg[:], u16[:], 1.0, wt[:, g * G:(g + 1) * G],
                    A.mult, A.mult, accum_out=e1cols[:, g:g + 1])

        bases = []
        b = 0
        for SGn in sg_sizes:
            bases.append(b)
            b += SGn
        for sg in range(len(sg_sizes)):
            emit_ss(sg, bases[sg], sg_sizes[sg])
            emit_m(sg, bases[sg], sg_sizes[sg])

        e1t = rpool.tile([128, 1], F32)
        nc.vector.tensor_scalar(junkw[:], e1cols[:], 1.0, 0.0, A.mult,
                                A.add, accum_out=e1t[:])
        acc = rpool.tile([C, D], F32)
        nc.vector.tensor_copy(acc[:], ps[:])
        nc.sync.dma_start(out[:], acc[:])
        nc.sync.dma_start(oute[:], e1t[:])
    nc.compile()
    return nc


def _get_nc():
    if "nc" not in _cache:
        _cache["nc"] = _build_nc()
    return _cache["nc"]


def kernel(logits_clean, logits_aug1, logits_aug2, labels):
    import os

    from concourse.bass_utils import run_bass_kernel_spmd

    x3 = np.concatenate(
        [np.asarray(logits_clean, dtype=np.float32),
         np.asarray(logits_aug1, dtype=np.float32),
         np.asarray(logits_aug2, dtype=np.float32)], axis=0)
    lab1 = np.asarray(labels).astype(np.int64)
    lab3 = np.concatenate([lab1, lab1, lab1])
    counts = np.bincount(lab3, minlength=C).astype(np.float64)

    # [8, 128, T*D] tile-transposed fp16: partition p of core c holds row
    # c*R + t*128 + p of tile t at columns [t*D, (t+1)*D).
    xt = x3.reshape(N_CORES, T, 128, D).transpose(0, 2, 1, 3)
    xt = np.ascontiguousarray(xt.astype(np.float16).reshape(N_CORES, 128, T * D))
    labt = lab3.reshape(N_CORES, T, 128).transpose(0, 2, 1)  # [8,128,T]
    oh = (labt[..., None] == np.arange(C, dtype=np.int64)).astype(np.float16)
    oh = np.ascontiguousarray(oh.reshape(N_CORES, 128, T * C))
    wrow = (1.0 / np.maximum(counts, 1.0)).astype(np.float32)[labt]  # [8,128,T]
    wrow = np.ascontiguousarray(wrow)

    in_maps = []
    for c in range(N_CORES):
        in_maps.append({"xin": xt[c], "ohin": oh[c], "win": wrow[c]})

    nc = _get_nc()
    trace = bool(int(os.environ.get("KERNEL_TRACE", "0")))
    kw = {}
    if trace:
        kw = dict(trace=True, tmpdir=os.environ.get("KERNEL_TRACE_DIR"))
    br = run_bass_kernel_spmd(nc, in_maps, list(range(N_CORES)), **kw)
    _cache["last_results"] = br

    seg16 = np.zeros((C, D), np.float64)
    E1 = 0.0
    for c in range(N_CORES):
        seg16 += br.results[c]["acc"].astype(np.float64)
        E1 += float(br.results[c]["e1"].astype(np.float64).sum())

    seg = seg16 / 16.0                 # sum_{i in c} p_i (per dim)
    cnt = np.maximum(counts, 1.0)
    mix = seg / cnt[:, None]
    lm = np.log(np.maximum(mix, 1e-7))
    spw = (seg.sum(1) / cnt).sum()     # sum_i w_i * sum_d p_id
    T1 = E1 / 16.0 - LOG16 * spw
    T2 = ((seg * lm).sum(1) / cnt).sum()
    loss = (T1 - T2) / D
    return np.float32(0.01 * loss)


# revision 38
# speedup vs baseline: 1.0219x; 1.0219x over previous
"""SupJSD / ContrastiveLossPlus loss kernel for 8 Trainium2 NeuronCores.

v3: fp16 data path; bn_stats for row norms; scale-free batched Ln(x);
amat (one-hot * s16) built on the Scalar engine; single fp16 matmul/tile.

Data arrives fp16, tile-transposed, in 16-tile groups (8KB/partition DMAs).
Per 128-row tile k (x >= 0, uniform [0,1)):
  bn_stats(x_k)  -> per-row (mean, var) of even/odd halves  (DVE, no accum)
    ss = cvar_e+cvar_o + 128*(me^2+mo^2)   sx = 128*(me+mo)   [group smalls]
  s16  = exp(-0.5*ln(ss)+ln16) = 16/||x||  (ACT Ln+Exp, batched /supergroup)
  lnx  = ln(x + 1e-30)                     (ACT Ln, batched per group,
                                            independent of the s16 chain)
  xlx_k = sum_d x*lnx                      (DVE stt accum)
  amat_k = onehot_k * s16                  (ACT Copy with per-row scale)
  psum += amat_k^T @ x                     (PE fp16 matmul -> 16*seg [80,256])
  u16 = s16*(xlx + (-0.5*ln(ss)+ln16)*sx)  [= 16*sum_d p*ln(16p), smalls]
End: e1 = sum_k u16[:,k]*W[:,k]  (W = 1/count per row, from host).
Host combines in float64:
  T1 = E1/16 - ln16 * sum_c (sum_d seg)/cnt
  loss = 0.01/D * (T1 - sum_c (1/cnt) sum_d seg*ln(clip(seg/cnt,eps)))
"""

import numpy as np

N_CORES = 8
N, D, C = 65536, 256, 80
R = 3 * N // N_CORES          # rows per core = 24576
T = R // 128                  # tiles per core = 192
G = 32                        # tiles per group (batched Ln)
NG = T // G                   # groups = 12
SG = 2                        # groups per supergroup (batched s16)
LOG16 = float(np.log(16.0))

_cache = {}


def _build_nc():
    from contextlib import ExitStack

    import concourse.tile as tile
    from concourse import bacc, mybir

    F32 = mybir.dt.float32
    F16 = mybir.dt.float16
    A = mybir.AluOpType
    ACTF = mybir.ActivationFunctionType

    nc = bacc.Bacc("TRN2", target_bir_lowering=False, debug=False,
                   num_devices=N_CORES)
    xin = nc.dram_tensor("xin", [128, T * D], F16, kind="ExternalInput").ap()
    ohin = nc.dram_tensor("ohin", [128, T * C], F16, kind="ExternalInput").ap()
    win = nc.dram_tensor("win", [128, T], F32, kind="ExternalInput").ap()
    out = nc.dram_tensor("acc", [C, D], F32, kind="ExternalOutput").ap()
    oute = nc.dram_tensor("e1", [128, 1], F32, kind="ExternalOutput").ap()

    with tile.TileContext(nc) as tc, ExitStack() as ctx:
        cpool = ctx.enter_context(tc.tile_pool(name="consts", bufs=1))
        xpool = ctx.enter_context(tc.tile_pool(name="x", bufs=4))
        mpool = ctx.enter_context(tc.tile_pool(name="m", bufs=3))
        lpool = ctx.enter_context(tc.tile_pool(name="lnm", bufs=4))
        opool = ctx.enter_context(tc.tile_pool(name="oh", bufs=NG + 1))
        spool = ctx.enter_context(tc.tile_pool(name="small", bufs=3))
        jpool = ctx.enter_context(tc.tile_pool(name="junk", bufs=2))
        rpool = ctx.enter_context(tc.tile_pool(name="res", bufs=1))
        pspool = ctx.enter_context(tc.tile_pool(name="ps", bufs=1, space="PSUM"))

        c_tiny24 = cpool.tile([128, 1], F32)
        nc.vector.memset(c_tiny24[:], 1e-24)
        c_tiny30 = cpool.tile([128, 1], F32)
        nc.vector.memset(c_tiny30[:], 1e-30)
        c_ln16 = cpool.tile([128, 1], F32)
        nc.vector.memset(c_ln16[:], LOG16)

        wt = cpool.tile([128, T], F32)
        nc.sync.dma_start(wt[:], win[:])

        e1cols = cpool.tile([128, NG], F32)
        junk1 = jpool.tile([128, D], F16, tag="junk")
        junk2 = jpool.tile([128, D], F16, tag="junk")
        junkg = jpool.tile([128, G], F32, tag="junkg")
        junkw = jpool.tile([128, NG], F32, tag="junkw")

        ps = pspool.tile([C, D], F32)

        xga, ohga = [], []
        for g in range(NG):
            xg = xpool.tile([128, G * D], F16, tag="xg")
            nch = 4 if g == 0 else 2
            W = G * D // nch
            for h in range(nch):
                nc.sync.dma_start(
                    xg[:, h * W:(h + 1) * W],
                    xin[:, g * G * D + h * W:g * G * D + (h + 1) * W])
            ohg = opool.tile([128, G * C], F16, tag="ohg")
            nc.sync.dma_start(ohg[:], ohin[:, g * G * C:(g + 1) * G * C])
            xga.append(xg)
            ohga.append(ohg)

        sg_sizes = [2, 2, 2]
        assert sum(sg_sizes) == NG
        sg_state = {}

        def emit_ss(sg, g_base, SGn):
            ssg = spool.tile([128, SG * G], F32, tag="ssg")
            sxg = spool.tile([128, SG * G], F32, tag="sxg")
            lnxs = []
            for g4 in range(SGn):
                g = g_base + g4
                xg = xga[g]
                # bn_stats per tile: (cnt,mean,cnt*var) of even/odd element
                # halves in one DVE pass each, no accumulator read needed.
                st = spool.tile([128, G * 6], F32, tag="bnst")
                for j in range(G):
                    nc.vector.bn_stats(st[:, j * 6:(j + 1) * 6],
                                       xg[:, j * D:(j + 1) * D])
                # lnx = ln(x + 1e-30), batched; independent of the s16 chain
                lnxg = lpool.tile([128, G * D], F16, tag="lnxg")
                H = G * D // 2
                nc.scalar.activation(lnxg[:, 0:H], xg[:, 0:H], ACTF.Ln,
                                     bias=c_tiny30[:])
                nc.scalar.activation(lnxg[:, H:], xg[:, H:], ACTF.Ln,
                                     bias=c_tiny30[:])
                lnxs.append(lnxg)
                # ss = cvar_e + cvar_o + (D/2)*(mean_e^2+mean_o^2); sx/128 = me+mo
                me, ve = st[:, 1::6], st[:, 2::6]
                mo, vo = st[:, 4::6], st[:, 5::6]
                sl = slice(g4 * G, (g4 + 1) * G)
                nc.vector.tensor_tensor(sxg[:, sl], me, mo, A.add)
                t1 = spool.tile([128, G], F32, tag="t1m")
                nc.vector.tensor_tensor(t1[:], me, me, A.mult)
                t2 = spool.tile([128, G], F32, tag="t2m")
                nc.vector.tensor_tensor(t2[:], mo, mo, A.mult)
                r = spool.tile([128, G], F32, tag="rm")
                nc.vector.tensor_tensor(r[:], ve, vo, A.add)
                t3 = spool.tile([128, G], F32, tag="t3m")
                nc.vector.scalar_tensor_tensor(
                    t3[:], t1[:], 1.0, t2[:], A.mult, A.add)
                nc.vector.scalar_tensor_tensor(
                    ssg[:, sl], t3[:], float(D // 2), r[:], A.mult, A.add)
            # s16 = exp(-0.5*ln(ss + 1e-24) + ln16) = 16/sqrt(ss)
            lss = spool.tile([128, SG * G], F32, tag="lss")
            nc.scalar.activation(lss[:, 0:SGn * G], ssg[:, 0:SGn * G],
                                 ACTF.Ln, bias=c_tiny24[:])
            s16g = spool.tile([128, SG * G], F32, tag="s16g")
            nc.scalar.activation(s16g[:, 0:SGn * G], lss[:, 0:SGn * G],
                                 ACTF.Exp, bias=c_ln16[:], scale=-0.5)
            sg_state[sg] = (lnxs, sxg, lss, s16g)

        def emit_m(sg, g_base, SGn):
            lnxs, sxg, lss, s16g = sg_state.pop(sg)
            for g4 in range(SGn):
                g = g_base + g4
                xg, ohg, lnxg = xga[g], ohga[g], lnxs[g4]
                sl = slice(g4 * G, (g4 + 1) * G)
                # amat = onehot * s16 (per-row), built on the Scalar engine
                am = mpool.tile([128, G * C], F16, tag="am")
                for j in range(G):
                    s16c = s16g[:, g4 * G + j:g4 * G + j + 1]
                    if g == NG - 1:
                        nc.vector.tensor_scalar(
                            am[:, j * C:(j + 1) * C],
                            ohg[:, j * C:(j + 1) * C], s16c, None, A.mult)
                    else:
                        nc.scalar.activation(
                            am[:, j * C:(j + 1) * C], ohg[:, j * C:(j + 1) * C],
                            ACTF.Copy, scale=s16c)

                ug = spool.tile([128, G], F32, tag="ug")
                for j in range(G):
                    k = g * G + j
                    nc.vector.scalar_tensor_tensor(
                        junk2[:], xg[:, j * D:(j + 1) * D], 1.0,
                        lnxg[:, j * D:(j + 1) * D], A.mult, A.mult,
                        accum_out=ug[:, j:j + 1])
                    nc.tensor.matmul(ps[:], am[:, j * C:(j + 1) * C],
                                     xg[:, j * D:(j + 1) * D],
                                     start=(k == 0), stop=(k == T - 1))
                # u16 = s16*(xlx + lns16*sx); lns16 = -0.5*lss + ln16
                lns = spool.tile([128, G], F32, tag="lns")
                nc.vector.tensor_scalar(lns[:], lss[:, sl], -0.5, LOG16,
                                        A.mult, A.add)
                a1 = spool.tile([128, G], F32, tag="a1m")
                nc.vector.tensor_tensor(a1[:], lns[:], sxg[:, sl], A.mult)
                b1 = spool.tile([128, G], F32, tag="b1m")
                nc.vector.scalar_tensor_tensor(
                    b1[:], a1[:], float(D // 2), ug[:], A.mult, A.add)
                u16 = spool.tile([128, G], F32, tag="u16m")
                nc.vector.tensor_tensor(u16[:], b1[:], s16g[:, sl], A.mult)
                nc.vector.scalar_tensor_tensor(
                    junkg[:], u16[:], 1.0, wt[:, g * G:(g + 1) * G],
                    A.mult, A.mult, accum_out=e1cols[:, g:g + 1])

        bases = []
        b = 0
        for SGn in sg_sizes:
            bases.append(b)
            b += SGn
        for sg in range(len(sg_sizes)):
            emit_ss(sg, bases[sg], sg_sizes[sg])
            emit_m(sg, bases[sg], sg_sizes[sg])

        e1t = rpool.tile([128, 1], F32)
        nc.vector.tensor_scalar(junkw[:], e1cols[:], 1.0, 0.0, A.mult,
                                A.add, accum_out=e1t[:])
        acc = rpool.tile([C, D], F32)
        nc.vector.tensor_copy(acc[:], ps[:])
        nc.sync.dma_start(out[:], acc[:])
        nc.sync.dma_start(oute[:], e1t[:])
    nc.compile()
    return nc


def _get_nc():
    if "nc" not in _cache:
        _cache["nc"] = _build_nc()
    return _cache["nc"]


def kernel(logits_clean, logits_aug1, logits_aug2, labels):
    import os

    from concourse.bass_utils import run_bass_kernel_spmd

    x3 = np.concatenate(
        [np.asarray(logits_clean, dtype=np.float32),
         np.asarray(logits_aug1, dtype=np.float32),
         np.asarray(logits_aug2, dtype=np.float32)], axis=0)
    lab1 = np.asarray(labels).astype(np.int64)
    lab3 = np.concatenate([lab1, lab1, lab1])
    counts = np.bincount(lab3, minlength=C).astype(np.float64)

    # [8, 128, T*D] tile-transposed fp16: partition p of core c holds row
    # c*R + t*128 + p of tile t at columns [t*D, (t+1)*D).
    xt = x3.reshape(N_CORES, T, 128, D).transpose(0, 2, 1, 3)
    xt = np.ascontiguousarray(xt.astype(np.float16).reshape(N_CORES, 128, T * D))
    labt = lab3.reshape(N_CORES, T, 128).transpose(0, 2, 1)  # [8,128,T]
    oh = (labt[..., None] == np.arange(C, dtype=np.int64)).astype(np.float16)
    oh = np.ascontiguousarray(oh.reshape(N_CORES, 128, T * C))
    wrow = (1.0 / np.maximum(counts, 1.0)).astype(np.float32)[labt]  # [8,128,T]
    wrow = np.ascontiguousarray(wrow)

    in_maps = []
    for c in range(N_CORES):
        in_maps.append({"xin": xt[c], "ohin": oh[c], "win": wrow[c]})

    nc = _get_nc()
    trace = bool(int(os.environ.get("KERNEL_TRACE", "0")))
    kw = {}
    if trace:
        kw = dict(trace=True, tmpdir=os.environ.get("KERNEL_TRACE_DIR"))
    br = run_bass_kernel_spmd(nc, in_maps, list(range(N_CORES)), **kw)
    _cache["last_results"] = br

    seg16 = np.zeros((C, D), np.float64)
    E1 = 0.0
    for c in range(N_CORES):
        seg16 += br.results[c]["acc"].astype(np.float64)
        E1 += float(br.results[c]["e1"].astype(np.float64).sum())

    seg = seg16 / 16.0                 # sum_{i in c} p_i (per dim)
    cnt = np.maximum(counts, 1.0)
    mix = seg / cnt[:, None]
    lm = np.log(np.maximum(mix, 1e-7))
    spw = (seg.sum(1) / cnt).sum()     # sum_i w_i * sum_d p_id
    T1 = E1 / 16.0 - LOG16 * spw
    T2 = ((seg * lm).sum(1) / cnt).sum()
    loss = (T1 - T2) / D
    return np.float32(0.01 * loss)


# revision 39
# speedup vs baseline: 1.0230x; 1.0011x over previous
"""SupJSD / ContrastiveLossPlus loss kernel for 8 Trainium2 NeuronCores.

v3: fp16 data path; bn_stats for row norms; scale-free batched Ln(x);
amat (one-hot * s16) built on the Scalar engine; single fp16 matmul/tile.

Data arrives fp16, tile-transposed, in 16-tile groups (8KB/partition DMAs).
Per 128-row tile k (x >= 0, uniform [0,1)):
  bn_stats(x_k)  -> per-row (mean, var) of even/odd halves  (DVE, no accum)
    ss = cvar_e+cvar_o + 128*(me^2+mo^2)   sx = 128*(me+mo)   [group smalls]
  s16  = exp(-0.5*ln(ss)+ln16) = 16/||x||  (ACT Ln+Exp, batched /supergroup)
  lnx  = ln(x + 1e-30)                     (ACT Ln, batched per group,
                                            independent of the s16 chain)
  xlx_k = sum_d x*lnx                      (DVE stt accum)
  amat_k = onehot_k * s16                  (ACT Copy with per-row scale)
  psum += amat_k^T @ x                     (PE fp16 matmul -> 16*seg [80,256])
  u16 = s16*(xlx + (-0.5*ln(ss)+ln16)*sx)  [= 16*sum_d p*ln(16p), smalls]
End: e1 = sum_k u16[:,k]*W[:,k]  (W = 1/count per row, from host).
Host combines in float64:
  T1 = E1/16 - ln16 * sum_c (sum_d seg)/cnt
  loss = 0.01/D * (T1 - sum_c (1/cnt) sum_d seg*ln(clip(seg/cnt,eps)))
"""

import numpy as np

N_CORES = 8
N, D, C = 65536, 256, 80
R = 3 * N // N_CORES          # rows per core = 24576
T = R // 128                  # tiles per core = 192
G = 32                        # tiles per group (batched Ln)
NG = T // G                   # groups = 12
SG = 2                        # groups per supergroup (batched s16)
LOG16 = float(np.log(16.0))

_cache = {}


def _build_nc():
    from contextlib import ExitStack

    import concourse.tile as tile
    from concourse import bacc, mybir

    F32 = mybir.dt.float32
    F16 = mybir.dt.float16
    A = mybir.AluOpType
    ACTF = mybir.ActivationFunctionType

    nc = bacc.Bacc("TRN2", target_bir_lowering=False, debug=False,
                   num_devices=N_CORES)
    xin = nc.dram_tensor("xin", [128, T * D], F16, kind="ExternalInput").ap()
    ohin = nc.dram_tensor("ohin", [128, T * C], F16, kind="ExternalInput").ap()
    win = nc.dram_tensor("win", [128, T], F32, kind="ExternalInput").ap()
    out = nc.dram_tensor("acc", [C, D], F32, kind="ExternalOutput").ap()
    oute = nc.dram_tensor("e1", [128, 1], F32, kind="ExternalOutput").ap()

    with tile.TileContext(nc) as tc, ExitStack() as ctx:
        cpool = ctx.enter_context(tc.tile_pool(name="consts", bufs=1))
        xpool = ctx.enter_context(tc.tile_pool(name="x", bufs=4))
        mpool = ctx.enter_context(tc.tile_pool(name="m", bufs=3))
        lpool = ctx.enter_context(tc.tile_pool(name="lnm", bufs=4))
        opool = ctx.enter_context(tc.tile_pool(name="oh", bufs=NG + 1))
        spool = ctx.enter_context(tc.tile_pool(name="small", bufs=3))
        jpool = ctx.enter_context(tc.tile_pool(name="junk", bufs=2))
        rpool = ctx.enter_context(tc.tile_pool(name="res", bufs=1))
        pspool = ctx.enter_context(tc.tile_pool(name="ps", bufs=1, space="PSUM"))

        c_tiny24 = cpool.tile([128, 1], F32)
        nc.vector.memset(c_tiny24[:], 1e-24)
        c_tiny30 = cpool.tile([128, 1], F32)
        nc.vector.memset(c_tiny30[:], 1e-30)
        c_ln16 = cpool.tile([128, 1], F32)
        nc.vector.memset(c_ln16[:], LOG16)

        wt = cpool.tile([128, T], F32)
        nc.sync.dma_start(wt[:], win[:])

        e1cols = cpool.tile([128, NG], F32)
        junk1 = jpool.tile([128, D], F16, tag="junk")
        junk2 = jpool.tile([128, D], F16, tag="junk")
        junkg = jpool.tile([128, G], F32, tag="junkg")
        junkw = jpool.tile([128, NG], F32, tag="junkw")

        ps = pspool.tile([C, D], F32)

        xga, ohga = [], []
        for g in range(NG):
            xg = xpool.tile([128, G * D], F16, tag="xg")
            nch = 4 if g == 0 else 2
            W = G * D // nch
            for h in range(nch):
                nc.sync.dma_start(
                    xg[:, h * W:(h + 1) * W],
                    xin[:, g * G * D + h * W:g * G * D + (h + 1) * W])
            ohg = opool.tile([128, G * C], F16, tag="ohg")
            nc.sync.dma_start(ohg[:], ohin[:, g * G * C:(g + 1) * G * C])
            xga.append(xg)
            ohga.append(ohg)

        sg_sizes = [2, 2, 2]
        assert sum(sg_sizes) == NG
        sg_state = {}

        def emit_ss(sg, g_base, SGn):
            ssg = spool.tile([128, SG * G], F32, tag="ssg")
            sxg = spool.tile([128, SG * G], F32, tag="sxg")
            lnxs = []
            for g4 in range(SGn):
                g = g_base + g4
                xg = xga[g]
                # bn_stats per tile: (cnt,mean,cnt*var) of even/odd element
                # halves in one DVE pass each, no accumulator read needed.
                st = spool.tile([128, G * 6], F32, tag="bnst")
                for j in range(G):
                    nc.vector.bn_stats(st[:, j * 6:(j + 1) * 6],
                                       xg[:, j * D:(j + 1) * D])
                # lnx = ln(x + 1e-30), batched; independent of the s16 chain
                lnxg = lpool.tile([128, G * D], F16, tag="lnxg")
                H = G * D // 2
                nc.scalar.activation(lnxg[:, 0:H], xg[:, 0:H], ACTF.Ln,
                                     bias=c_tiny30[:])
                nc.scalar.activation(lnxg[:, H:], xg[:, H:], ACTF.Ln,
                                     bias=c_tiny30[:])
                lnxs.append(lnxg)
                # ss = cvar_e + cvar_o + (D/2)*(mean_e^2+mean_o^2); sx/128 = me+mo
                me, ve = st[:, 1::6], st[:, 2::6]
                mo, vo = st[:, 4::6], st[:, 5::6]
                sl = slice(g4 * G, (g4 + 1) * G)
                nc.vector.tensor_tensor(sxg[:, sl], me, mo, A.add)
                t1 = spool.tile([128, G], F32, tag="t1m")
                nc.vector.tensor_tensor(t1[:], me, me, A.mult)
                t2 = spool.tile([128, G], F32, tag="t2m")
                nc.vector.tensor_tensor(t2[:], mo, mo, A.mult)
                r = spool.tile([128, G], F32, tag="rm")
                nc.vector.tensor_tensor(r[:], ve, vo, A.add)
                t3 = spool.tile([128, G], F32, tag="t3m")
                nc.vector.scalar_tensor_tensor(
                    t3[:], t1[:], 1.0, t2[:], A.mult, A.add)
                nc.vector.scalar_tensor_tensor(
                    ssg[:, sl], t3[:], float(D // 2), r[:], A.mult, A.add)
            # s16 = exp(-0.5*ln(ss + 1e-24) + ln16) = 16/sqrt(ss)
            lss = spool.tile([128, SG * G], F32, tag="lss")
            nc.scalar.activation(lss[:, 0:SGn * G], ssg[:, 0:SGn * G],
                                 ACTF.Ln, bias=c_tiny24[:])
            s16g = spool.tile([128, SG * G], F32, tag="s16g")
            nc.scalar.activation(s16g[:, 0:SGn * G], lss[:, 0:SGn * G],
                                 ACTF.Exp, bias=c_ln16[:], scale=-0.5)
            sg_state[sg] = (lnxs, sxg, lss, s16g)

        def emit_m(sg, g_base, SGn):
            lnxs, sxg, lss, s16g = sg_state.pop(sg)
            for g4 in range(SGn):
                g = g_base + g4
                xg, ohg, lnxg = xga[g], ohga[g], lnxs[g4]
                sl = slice(g4 * G, (g4 + 1) * G)
                # amat = onehot * s16 (per-row), built on the Scalar engine
                am = mpool.tile([128, G * C], F16, tag="am")
                for j in range(G):
                    s16c = s16g[:, g4 * G + j:g4 * G + j + 1]
                    nc.scalar.activation(
                        am[:, j * C:(j + 1) * C], ohg[:, j * C:(j + 1) * C],
                        ACTF.Copy, scale=s16c)

                ug = spool.tile([128, G], F32, tag="ug")
                for j in range(G):
                    k = g * G + j
                    nc.vector.scalar_tensor_tensor(
                        junk2[:], xg[:, j * D:(j + 1) * D], 1.0,
                        lnxg[:, j * D:(j + 1) * D], A.mult, A.mult,
                        accum_out=ug[:, j:j + 1])
                    nc.tensor.matmul(ps[:], am[:, j * C:(j + 1) * C],
                                     xg[:, j * D:(j + 1) * D],
                                     start=(k == 0), stop=(k == T - 1))
                # u16 = s16*(xlx + lns16*sx); lns16 = -0.5*lss + ln16
                lns = spool.tile([128, G], F32, tag="lns")
                nc.vector.tensor_scalar(lns[:], lss[:, sl], -0.5, LOG16,
                                        A.mult, A.add)
                a1 = spool.tile([128, G], F32, tag="a1m")
                nc.vector.tensor_tensor(a1[:], lns[:], sxg[:, sl], A.mult)
                b1 = spool.tile([128, G], F32, tag="b1m")
                nc.vector.scalar_tensor_tensor(
                    b1[:], a1[:], float(D // 2), ug[:], A.mult, A.add)
                u16 = spool.tile([128, G], F32, tag="u16m")
                nc.vector.tensor_tensor(u16[:], b1[:], s16g[:, sl], A.mult)
                nc.vector.scalar_tensor_tensor(
                    junkg[:], u16[:], 1.0, wt[:, g * G:(g + 1) * G],
                    A.mult, A.mult, accum_out=e1cols[:, g:g + 1])

        bases = []
        b = 0
        for SGn in sg_sizes:
            bases.append(b)
            b += SGn
        for sg in range(len(sg_sizes)):
            emit_ss(sg, bases[sg], sg_sizes[sg])
            emit_m(sg, bases[sg], sg_sizes[sg])

        e1t = rpool.tile([128, 1], F32)
        nc.vector.tensor_scalar(junkw[:], e1cols[:], 1.0, 0.0, A.mult,
                                A.add, accum_out=e1t[:])
        acc = rpool.tile([C, D], F32)
        nc.vector.tensor_copy(acc[:], ps[:])
        nc.sync.dma_start(out[:], acc[:])
        nc.sync.dma_start(oute[:], e1t[:])
    nc.compile()
    return nc


def _get_nc():
    if "nc" not in _cache:
        _cache["nc"] = _build_nc()
    return _cache["nc"]


def kernel(logits_clean, logits_aug1, logits_aug2, labels):
    import os

    from concourse.bass_utils import run_bass_kernel_spmd

    x3 = np.concatenate(
        [np.asarray(logits_clean, dtype=np.float32),
         np.asarray(logits_aug1, dtype=np.float32),
         np.asarray(logits_aug2, dtype=np.float32)], axis=0)
    lab1 = np.asarray(labels).astype(np.int64)
    lab3 = np.concatenate([lab1, lab1, lab1])
    counts = np.bincount(lab3, minlength=C).astype(np.float64)

    # [8, 128, T*D] tile-transposed fp16: partition p of core c holds row
    # c*R + t*128 + p of tile t at columns [t*D, (t+1)*D).
    xt = x3.reshape(N_CORES, T, 128, D).transpose(0, 2, 1, 3)
    xt = np.ascontiguousarray(xt.astype(np.float16).reshape(N_CORES, 128, T * D))
    labt = lab3.reshape(N_CORES, T, 128).transpose(0, 2, 1)  # [8,128,T]
    oh = (labt[..., None] == np.arange(C, dtype=np.int64)).astype(np.float16)
    oh = np.ascontiguousarray(oh.reshape(N_CORES, 128, T * C))
    wrow = (1.0 / np.maximum(counts, 1.0)).astype(np.float32)[labt]  # [8,128,T]
    wrow = np.ascontiguousarray(wrow)

    in_maps = []
    for c in range(N_CORES):
        in_maps.append({"xin": xt[c], "ohin": oh[c], "win": wrow[c]})

    nc = _get_nc()
    trace = bool(int(os.environ.get("KERNEL_TRACE", "0")))
    kw = {}
    if trace:
        kw = dict(trace=True, tmpdir=os.environ.get("KERNEL_TRACE_DIR"))
    br = run_bass_kernel_spmd(nc, in_maps, list(range(N_CORES)), **kw)
    _cache["last_results"] = br

    seg16 = np.zeros((C, D), np.float64)
    E1 = 0.0
    for c in range(N_CORES):
        seg16 += br.results[c]["acc"].astype(np.float64)
        E1 += float(br.results[c]["e1"].astype(np.float64).sum())

    seg = seg16 / 16.0                 # sum_{i in c} p_i (per dim)
    cnt = np.maximum(counts, 1.0)
    mix = seg / cnt[:, None]
    lm = np.log(np.maximum(mix, 1e-7))
    spw = (seg.sum(1) / cnt).sum()     # sum_i w_i * sum_d p_id
    T1 = E1 / 16.0 - LOG16 * spw
    T2 = ((seg * lm).sum(1) / cnt).sum()
    loss = (T1 - T2) / D
    return np.float32(0.01 * loss)
